# revision 1
# baseline (speedup 1.0000x reference)
"""Causal GQA self-attention on 8 Trainium2 NeuronCores.

Sharding: data-parallel over batch (4) x tensor-parallel over heads (2 halves
of 14 heads each, KV heads replicated for the shared GQA group). Each core
computes a partial output (its heads' contribution through the row-parallel
out-projection); the host sums the two partials per batch element.

Per-core head assignment is chosen so every core sees an identical local
structure (local heads 0..13, local kv-groups 0..3, quad q <-> group q):
  half 0: global heads [0..11, 24, 25],  kv heads [0, 1, 2, 6]
  half 1: global heads [12..23, 26, 27], kv heads [3, 4, 5, 6]
The host permutes weight columns/rows into this local order.

Kernel layout strategy (all SBUF tensors [128 partitions, free...]):
  xT  [128, 7, 2048] : x^T (C on partitions) via PE transpose
  QT  [128, 4, 2048] : Q^T, local head h at (partitions 32*(h%4), chunk h//4)
  KT  [128, 4, 2048] : K^T per local group, replicated on all 4 row slots
  V   [128, 16, 128] : V (kpos on partitions)
  AOT [128, 4, 2048] : attention output transposed (head dims on partitions)
Scores are computed transposed S^T[kpos, q] with 4 row-tiled (tile_position)
K=32 matmuls per quad; exp on ScalarE (PSUM->SBUF, scale folded in); P^T then
feeds col-tiled AV and Z(=sum) matmuls accumulating over kpos chunks; final
out-projection consumes AOT directly as the stationary operand.
"""

import sys

sys.path.insert(0, "/opt/trn_rl_repo")

from contextlib import ExitStack

import numpy as np

import concourse.bass as bass
import concourse.mybir as mybir
import concourse.tile as tile
from concourse import bacc
from concourse.bass import ts
from concourse.bass_utils import run_bass_kernel_spmd

F32 = mybir.dt.float32
F32R = mybir.dt.float32r
EXP = mybir.ActivationFunctionType.Exp
P = 128
T, C = 2048, 896
D = 32
HL = 14  # local heads per core
GL = 4  # local kv groups per core
DH = HL * D  # 448
DKV = GL * D  # 128
SCALE = 1.0 / float(np.sqrt(D))

HEADS_HALF = [
    list(range(0, 12)) + [24, 25],
    list(range(12, 24)) + [26, 27],
]
KV_HALF = [[0, 1, 2, 6], [3, 4, 5, 6]]


def _trace(tc, d):
    nc = tc.nc
    with ExitStack() as ctx:
        const = ctx.enter_context(tc.tile_pool(name="const", bufs=1))
        ident = const.tile([P, P], F32)
        nc.sync.dma_start(ident[:], d["ident"][:])
        maskb = const.tile([P, P], F32)
        nc.sync.dma_start(maskb[:], d["mask"][:])
        identr = const.tile([P, P], F32R)
        nc.sync.dma_start(identr[:], d["identr"][:])

        persist = ctx.enter_context(tc.tile_pool(name="persist", bufs=1))
        QT = persist.tile([P, 4, T], F32R, tag="QT")
        KT = persist.tile([P, 4, T], F32R, tag="KT")
        V = persist.tile([P, 16, GL, 64], F32R, tag="V")

        nc.sync.dma_start(
            V[:, :, :, D:64],
            d["vones"].rearrange("p (a b c) -> p a b c", a=16, b=GL),
        )

        with tc.tile_pool(name="ph01", bufs=1) as ph01:
            xT = ph01.tile([P, 7, T], F32R, tag="xT")
            # ------------- phase 0: x -> xT (PE transpose) -------------
            with tc.tile_pool(name="xraw", bufs=8) as xraw, \
                 tc.tile_pool(name="pst", bufs=2, space="PSUM") as pst:
                xv = d["x"].rearrange("(to ti) c -> ti to c", ti=P)
                for tcg in range(4):
                    xt4 = []
                    for k in range(4):
                        xtile = xraw.tile([P, C], F32, tag="xtile")
                        nc.sync.dma_start(xtile[:], xv[:, 4 * tcg + k, :])
                        xt4.append(xtile)
                    for cc in range(7):
                        ps = pst.tile([P, 512], F32, tag="tps")
                        for k in range(4):
                            nc.tensor.transpose(
                                ps[:, ts(k, P)], xt4[k][:, ts(cc, P)], ident[:]
                            )
                        nc.vector.tensor_copy(xT[:, cc, ts(tcg, 512)], ps[:])

            # ---------------- phase 1: projections ----------------
            with tc.tile_pool(name="w1", bufs=1) as w1, \
                 tc.tile_pool(name="vtt", bufs=2) as vtt, \
                 tc.tile_pool(name="pst2", bufs=2, space="PSUM") as pst2, \
                 tc.tile_pool(name="psp", bufs=2, space="PSUM") as psp:
                WqH = w1.tile([P, 7, DH], F32R, tag="WqH")
                nc.sync.dma_start(
                    WqH[:], d["wq"].rearrange("(co ci) n -> ci co n", ci=P)
                )
                WkR = w1.tile([P, 7, GL, P], F32R, tag="WkR")
                wkv = d["wk"].rearrange("(co ci) n -> ci co n", ci=P)
                for g in range(GL):
                    for i in range(4):
                        nc.sync.dma_start(
                            WkR[:, :, g, ts(i, D)], wkv[:, :, ts(g, D)]
                        )
                WvH = w1.tile([P, 7, DKV], F32R, tag="WvH")
                nc.sync.dma_start(
                    WvH[:], d["wv"].rearrange("(co ci) n -> ci co n", ci=P)
                )

                # QT: out[m=dim chunk, n=t] accumulate over C chunks
                for mc in range(4):
                    M = P if mc < 3 else 64
                    for nk in range(4):
                        ps = psp.tile([P, 512], F32, tag="pps")
                        for c in range(7):
                            nc.tensor.matmul(
                                ps[:M, :],
                                lhsT=WqH[:, c, mc * P : mc * P + M],
                                rhs=xT[:, c, ts(nk, 512)],
                                start=(c == 0),
                                stop=(c == 6),
                            )
                        nc.vector.tensor_copy(QT[:M, mc, ts(nk, 512)], ps[:M, :])
                # KT (replicated): per local group
                for g in range(GL):
                    for nk in range(4):
                        ps = psp.tile([P, 512], F32, tag="pps")
                        for c in range(7):
                            nc.tensor.matmul(
                                ps[:],
                                lhsT=WkR[:, c, g, :],
                                rhs=xT[:, c, ts(nk, 512)],
                                start=(c == 0),
                                stop=(c == 6),
                            )
                        nc.vector.tensor_copy(KT[:, g, ts(nk, 512)], ps[:])
                # VT then transpose to V
                for nk in range(4):
                    ps = psp.tile([P, 512], F32, tag="pps")
                    for c in range(7):
                        nc.tensor.matmul(
                            ps[:],
                            lhsT=WvH[:, c, :],
                            rhs=xT[:, c, ts(nk, 512)],
                            start=(c == 0),
                            stop=(c == 6),
                        )
                    vts = vtt.tile([P, 512], F32, tag="vts")
                    nc.vector.tensor_copy(vts[:], ps[:])
                    for k in range(4):
                        vps = pst2.tile([P, 512], F32, tag="tps")
                        nc.tensor.transpose(vps[:, :P], vts[:, ts(k, P)], ident[:])
                        nc.vector.tensor_copy(
                            V[:, nk * 4 + k, :, 0:D],
                            vps[:, :P].rearrange("p (g e) -> p g e", g=GL),
                        )

        # ---------------- phase 2+3: attention + out-proj ----------------
        with tc.tile_pool(name="w2", bufs=1) as w2, \
             tc.tile_pool(name="pts", bufs=2) as pts, \
             tc.tile_pool(name="ziP", bufs=2) as zip_, \
             tc.tile_pool(name="outs", bufs=2) as outs_p, \
             tc.tile_pool(name="pss", bufs=2, space="PSUM") as pss, \
             tc.tile_pool(name="psav", bufs=2, space="PSUM") as psav, \
             tc.tile_pool(name="pso", bufs=2, space="PSUM") as pso:
            AOT = w2.tile([P, 4, T], F32R, tag="AOT")
            WoH = w2.tile([P, 4, C], F32R, tag="WoH")
            nc.sync.dma_start(
                WoH[:, :3, :], d["wo"][: 3 * P, :].rearrange("(co ci) n -> ci co n", ci=P)
            )
            nc.sync.dma_start(WoH[:64, 3, :], d["wo"][3 * P :, :])
            ov = d["out"].rearrange("(to ti) c -> ti to c", ti=P)

            for qc in range(4):
                qs = qc * 512
                for pr in range(7):
                    h0 = 2 * pr
                    g = h0 // 4
                    j0 = h0 % 4
                    ava = psav.tile([64, 512], F32, tag="av")
                    avb = psav.tile([64, 512], F32, tag="av")
                    avs = [ava, avb]
                    nks = qs // P + 4
                    for ki in range(nks):
                        ks = ki * P
                        qoff = max(0, ks - qs)
                        pt = pts.tile([P, 2, 512], F32R, tag="pt")
                        sp = pss.tile([P, 2, 512], F32, tag="sp")
                        for j2 in range(2):
                            j = j0 + j2
                            nc.tensor.matmul(
                                sp[:, j2, qoff:512],
                                lhsT=KT[ts(j, D), g, ks : ks + P],
                                rhs=QT[ts(j, D), g, qs + qoff : qs + 512],
                                start=True,
                                stop=True,
                                tile_position=(j * D, 0),
                            )
                        nc.scalar.activation(
                            pt[:, :, qoff:512],
                            sp[:, :, qoff:512],
                            EXP,
                            scale=SCALE,
                        )
                        if ks >= qs:  # diagonal chunk: zero the triangle
                            nc.vector.tensor_tensor(
                                pt[:, :, qoff : qoff + P],
                                pt[:, :, qoff : qoff + P],
                                maskb[:, None, :].to_broadcast((P, 2, P)),
                                mybir.AluOpType.mult,
                            )
                        for j2 in range(2):
                            nc.tensor.matmul(
                                avs[j2][0:64, qoff:512],
                                lhsT=V[:, ki, g, 0:64],
                                rhs=pt[:, j2, qoff:512],
                                start=(ki == 0),
                                stop=(ki == nks - 1),
                                skip_group_check=True,
                            )
                    zq = pss.tile([P, 2, 512], F32, tag="sp")
                    for j2 in range(2):
                        h = h0 + j2
                        av = avs[j2]
                        zt = zip_.tile([64, 512], F32R, tag="zt")
                        nc.vector.tensor_copy(zt[D:64, :], av[D:64, :])
                        nc.tensor.matmul(
                            zq[0:D, j2, :],
                            lhsT=identr[D:64, D:64],
                            rhs=zt[D:64, :],
                            start=True,
                            stop=True,
                            tile_position=(D, 0),
                        )
                        zs = zip_.tile([D, 512], F32, tag="zs")
                        nc.vector.reciprocal_approx_fast(zs[:], zq[0:D, j2, :])
                        ao = zip_.tile([D, 512], F32R, tag="ao")
                        nc.vector.tensor_tensor(
                            ao[:],
                            av[0:D, :],
                            zs[:],
                            mybir.AluOpType.mult,
                        )
                        nc.sync.dma_start(
                            AOT[ts(h % 4, D), g, qs : qs + 512], ao[:]
                        )
                # out-projection for this q-chunk
                for tcl in range(4):
                    tg = qc * 4 + tcl
                    ob = outs_p.tile([P, C], F32, tag="ob")
                    for ncol in range(2):
                        po = pso.tile([P, 448], F32, tag="po")
                        for c in range(4):
                            K = P if c < 3 else 64
                            nc.tensor.matmul(
                                po[:],
                                lhsT=AOT[:K, c, qs + tcl * P : qs + (tcl + 1) * P],
                                rhs=WoH[:K, c, ncol * 448 : (ncol + 1) * 448],
                                start=(c == 0),
                                stop=(c == 3),
                            )
                        nc.vector.tensor_copy(ob[:, ncol * 448 : (ncol + 1) * 448], po[:])
                    nc.sync.dma_start(ov[:, tg, :], ob[:])


_NC_CACHE = None


def _build():
    global _NC_CACHE
    if _NC_CACHE is not None:
        return _NC_CACHE
    nc = bacc.Bacc("TRN2", target_bir_lowering=False, debug=False, num_devices=8)
    d = {
        "x": nc.dram_tensor("x", (T, C), F32, kind="ExternalInput"),
        "wq": nc.dram_tensor("wq", (C, DH), F32R, kind="ExternalInput"),
        "wk": nc.dram_tensor("wk", (C, DKV), F32R, kind="ExternalInput"),
        "wv": nc.dram_tensor("wv", (C, DKV), F32R, kind="ExternalInput"),
        "wo": nc.dram_tensor("wo", (DH, C), F32R, kind="ExternalInput"),
        "ident": nc.dram_tensor("ident", (P, P), F32, kind="ExternalInput"),
        "mask": nc.dram_tensor("mask", (P, P), F32, kind="ExternalInput"),
        "vones": nc.dram_tensor("vones", (P, 16 * GL * D), F32R, kind="ExternalInput"),
        "identr": nc.dram_tensor("identr", (P, P), F32R, kind="ExternalInput"),
        "out": nc.dram_tensor("out", (T, C), F32, kind="ExternalOutput"),

    }
    with tile.TileContext(nc) as tc:
        _trace(tc, {k: v[:] for k, v in d.items()})
    nc.compile()
    _NC_CACHE = nc
    return nc


def _in_maps(x, Wq, Wk, Wv, Wo):
    ident = np.eye(P, dtype=np.float32)
    vones = np.ones((P, 16 * GL * D), dtype=np.float32)
    maskb = (
        np.arange(P)[None, :] >= np.arange(P)[:, None]
    ).astype(np.float32)  # [kpos_p, q_j] valid when j >= p
    maps = []
    for c in range(8):
        b, hf = c // 2, c % 2
        hcols = np.concatenate([np.arange(32 * h, 32 * h + 32) for h in HEADS_HALF[hf]])
        kcols = np.concatenate([np.arange(32 * g, 32 * g + 32) for g in KV_HALF[hf]])
        maps.append(
            {
                "x": np.ascontiguousarray(x[b]),
                "wq": np.ascontiguousarray(Wq[:, hcols]),
                "wk": np.ascontiguousarray(Wk[:, kcols]),
                "wv": np.ascontiguousarray(Wv[:, kcols]),
                "wo": np.ascontiguousarray(Wo[hcols, :]),
                "ident": ident,
                "mask": maskb,
                "vones": vones,
                "identr": ident,
            }
        )
    return maps


def run(x, Wq, Wk, Wv, Wo, trace=False):
    nc = _build()
    res = run_bass_kernel_spmd(
        nc, _in_maps(x, Wq, Wk, Wv, Wo), core_ids=list(range(8)), trace=trace
    )
    outs = [r["out"] for r in res.results]
    final = np.empty((4, T, C), np.float32)
    for b in range(4):
        final[b] = outs[2 * b] + outs[2 * b + 1]
    return final, res


def kernel(x, Wq, Wk, Wv, Wo):
    x = np.asarray(x, dtype=np.float32)
    out, _ = run(
        x,
        np.asarray(Wq, np.float32),
        np.asarray(Wk, np.float32),
        np.asarray(Wv, np.float32),
        np.asarray(Wo, np.float32),
    )
    return out



# revision 9
# speedup vs baseline: 1.4130x; 1.4130x over previous
"""Causal GQA self-attention on 8 Trainium2 NeuronCores (fp16 pipeline).

Sharding: data-parallel over batch (4) x tensor-parallel over heads (2 halves
of 14 heads each, KV heads replicated per GQA group). Each core computes its
heads' partial contribution through the row-parallel out-projection; the host
sums the two fp16 partials per batch element in fp32.

Per-core local structure: 4 local kv groups g (sizes 4,4,4,2 heads), local
head h -> (g = h//4, s = h%4 slot). All tensors fp16 except PSUM (f32).

Layouts (SBUF [128 partitions, free...]):
  X16 [128, 16, 896]   x rows (t%128 on partitions)
  xT  [128, 7, 2048]   x^T (C on partitions) via PE transpose
  QT  [128, 4, 2048]   Q^T: head (g,s) at partitions 32g..32g+32, slot s
  KT  [128, 2048]      K^T: group g at partitions 32g..32g+32
  V   [128, 16, 4, 33] V rows (kpos%128 on partitions), col 32 = ones (Z)
  AO  [128, 4, 448]    attn out rows (q%128 on partitions) per 512-q round
  AOT [128, 4, 2048]   attn out transposed (head dims on partitions)

Pipeline: 4 rounds of 512 tokens; per round: x-transpose + Q/K/V projection,
then per head: S^T = K^T.T @ Q^T per 128-kpos chunk (diagonal chunks get a
-180 mask preloaded into PSUM via an extra matmul, then accumulate), exp on
ScalarE (or Schraudolph fast-exp on DVE for a fraction of chunks: bit-trick
y = s*A+B -> int16 -> reinterpret as fp16), then AV flipped: out[q,d] with
P^T chunk as stationary operand and [V | 1] as 33-wide moving operand so the
softmax denominator Z rides along as column 32. Normalize on DVE, transpose
AO via PE, row-parallel out-projection, fp16 partial out.
"""

import sys

sys.path.insert(0, "/opt/trn_rl_repo")

import numpy as np

import concourse.bass as bass
import concourse.mybir as mybir
import concourse.tile as tile
from concourse import bacc
from concourse.bass import ts
from concourse.bass_utils import run_bass_kernel_spmd

F32 = mybir.dt.float32
F16 = mybir.dt.float16
I16 = mybir.dt.int16
EXP = mybir.ActivationFunctionType.Exp
MULT = mybir.AluOpType.mult
ADD = mybir.AluOpType.add
P = 128
T, C = 2048, 896
D = 32
HL = 14          # local heads per core
DH = HL * D      # 448
SCALE = 1.0 / float(np.sqrt(D))
MASKVAL = -180.0
# Schraudolph fast-exp consts (fp16 bit trick): y = s*A + B as int16
A_S = SCALE * 1024.0 / float(np.log(2.0))
B_S = 15.0 * 1024.0 - 0.043 * 1024.0
# every SCHR_MOD-th exp pair runs on DVE via Schraudolph (0 = all exact/ACT)
SCHR_MOD = 0

SOFF = [0, 128, 256, 352]   # Wq col offset per slot
SLOTW = [128, 128, 96, 96]  # slot widths (s>=2 lack group 3)

HEADS_HALF = [
    list(range(0, 12)) + [24, 25],
    list(range(12, 24)) + [26, 27],
]
KV_HALF = [[0, 1, 2, 6], [3, 4, 5, 6]]


def _head_gs(h):
    return (h // 4, h % 4) if h < 12 else (3, h - 12)


def _trace(tc_, d):
    nc = tc_.nc
    pair_ctr = [0]

    with tc_.tile_pool(name="const", bufs=1) as const, \
         tc_.tile_pool(name="persist", bufs=1) as persist, \
         tc_.tile_pool(name="aop", bufs=2) as aop, \
         tc_.tile_pool(name="ptp", bufs=18) as ptp, \
         tc_.tile_pool(name="rzp", bufs=2) as rzp, \
         tc_.tile_pool(name="obp", bufs=2) as obp, \
         tc_.tile_pool(name="spp", bufs=2, space="PSUM") as spp, \
         tc_.tile_pool(name="avp", bufs=2, space="PSUM") as avp, \
         tc_.tile_pool(name="mxp", bufs=2, space="PSUM") as mxp:

        identh = const.tile([P, P], F16)
        nc.sync.dma_start(identh[:], d["identh"][:])
        maskc = const.tile([P, P], F16)
        nc.sync.dma_start(maskc[:], d["maskc"][:])

        X16 = persist.tile([P, 16, C], F16, tag="X16")
        xT = persist.tile([P, 7, T], F16, tag="xT")
        QT = persist.tile([P, 4, T], F16, tag="QT")
        KT = persist.tile([P, T], F16, tag="KT")
        V = persist.tile([P, 16, 4, 33], F16, tag="V")
        AOT = persist.tile([P, 4, T], F16, tag="AOT")
        WqH = persist.tile([P, 7, DH], F16, tag="WqH")
        WkH = persist.tile([P, 7, P], F16, tag="WkH")
        WvH = persist.tile([P, 7, P], F16, tag="WvH")
        WoH = persist.tile([P, 4, C], F16, tag="WoH")

        xv = d["x"].rearrange("(to ti) c -> ti to c", ti=P)
        ov = d["out"].rearrange("(to ti) c -> ti to c", ti=P)

        # input DMAs: round-0 x first, then QKV weights, rest of x, Wo
        nc.sync.dma_start(X16[:, 0:4, :], xv[:, 0:4, :])
        nc.sync.dma_start(WqH[:], d["wq"].rearrange("(co ci) n -> ci co n", ci=P))
        nc.sync.dma_start(WkH[:], d["wk"].rearrange("(co ci) n -> ci co n", ci=P))
        nc.sync.dma_start(WvH[:], d["wv"].rearrange("(co ci) n -> ci co n", ci=P))
        for r in range(1, 4):
            nc.sync.dma_start(X16[:, 4 * r:4 * r + 4, :], xv[:, 4 * r:4 * r + 4, :])
        nc.sync.dma_start(
            WoH[:, :3, :], d["wo"][:3 * P, :].rearrange("(co ci) n -> ci co n", ci=P)
        )
        nc.sync.dma_start(WoH[:64, 3, :], d["wo"][3 * P:, :])
        nc.gpsimd.memset(V[:, :, :, 32:33], 1.0)

        def emit_av(AO, h, qc, pts):
            # one contiguous accumulation chain per q-subblock j (PSUM banks
            # support a single open matmul accumulation group at a time)
            g, _ = _head_gs(h)
            for j in range(4):
                av = avp.tile([P, 33], F32, tag="av", name="av")
                for ki in range(4 * qc + j + 1):
                    nc.tensor.matmul(
                        av[:, 0:33],
                        lhsT=pts[ki // 2][:, ki % 2, ts(j, P)],
                        rhs=V[:, ki, g, 0:33],
                        start=(ki == 0),
                        stop=(ki == 4 * qc + j),
                        skip_group_check=True,
                    )
                rz = rzp.tile([P, 1], F32, tag="rz", name="rz")
                nc.vector.reciprocal_approx_fast(rz[:], av[:, 32:33])
                nc.vector.tensor_tensor(
                    AO[:, j, h * D:(h + 1) * D],
                    av[:, 0:D],
                    rz.to_broadcast((P, D)),
                    MULT,
                )

        def head_block(AO, h, qc):
            g, s = _head_gs(h)
            qs = qc * 512
            npair = 2 * qc + 2
            pts = []
            for p in range(npair):
                sp = spp.tile([P, 2, 512], F32, tag="sp")
                for sl in range(2):
                    ki = 2 * p + sl
                    ks = ki * P
                    dg = ki - 4 * qc
                    if dg < 0:
                        nc.tensor.matmul(
                            sp[:, sl, :],
                            lhsT=KT[ts(g, D), ks:ks + P],
                            rhs=QT[ts(g, D), s, qs:qs + 512],
                            start=True, stop=True, skip_group_check=True,
                            tile_position=(g * D, 0),
                        )
                    else:
                        qoff = P * dg
                        nc.tensor.matmul(
                            sp[:, sl, qoff:qoff + P],
                            lhsT=identh[:],
                            rhs=maskc[:],
                            start=True, stop=False, skip_group_check=True,
                        )
                        nc.tensor.matmul(
                            sp[:, sl, qoff:qoff + P],
                            lhsT=KT[ts(g, D), ks:ks + P],
                            rhs=QT[ts(g, D), s, qs + qoff:qs + qoff + P],
                            start=False, stop=True, skip_group_check=True,
                            tile_position=(g * D, 0),
                        )
                        if qoff + P < 512:
                            nc.tensor.matmul(
                                sp[:, sl, qoff + P:512],
                                lhsT=KT[ts(g, D), ks:ks + P],
                                rhs=QT[ts(g, D), s, qs + qoff + P:qs + 512],
                                start=True, stop=True, skip_group_check=True,
                                tile_position=(g * D, 0),
                            )
                qoffE = P * max(0, 2 * p - 4 * qc)
                pt = ptp.tile([P, 2, 512], F16, tag="pt")
                pair_ctr[0] += 1
                if SCHR_MOD and pair_ctr[0] % SCHR_MOD == 0:
                    nc.vector.tensor_scalar(
                        pt[:, :, qoffE:512].bitcast(I16),
                        sp[:, :, qoffE:512],
                        A_S, B_S, MULT, ADD,
                    )
                else:
                    nc.scalar.activation(
                        pt[:, :, qoffE:512], sp[:, :, qoffE:512], EXP, scale=SCALE
                    )
                pts.append(pt)
            return pts

        for tc in range(4):
            # ---- prep: x transpose + projections for this 512-token round
            for cc in range(7):
                pst = mxp.tile([P, 512], F16, tag="mx", name="pst")
                for k2 in range(4):
                    nc.tensor.transpose(
                        pst[:, ts(k2, P)], X16[:, 4 * tc + k2, ts(cc, P)], identh[:]
                    )
                nc.vector.tensor_copy(xT[:, cc, ts(tc, 512)], pst[:])
            for s in range(4):
                Ms = SLOTW[s]
                qp = mxp.tile([P, 512], F32, tag="mx", name="qp")
                for cc in range(7):
                    nc.tensor.matmul(
                        qp[0:Ms, :],
                        lhsT=WqH[:, cc, SOFF[s]:SOFF[s] + Ms],
                        rhs=xT[:, cc, ts(tc, 512)],
                        start=(cc == 0), stop=(cc == 6),
                    )
                nc.vector.tensor_copy(QT[0:Ms, s, ts(tc, 512)], qp[0:Ms, :])
            kp = mxp.tile([P, 512], F32, tag="mx", name="kp")
            for cc in range(7):
                nc.tensor.matmul(
                    kp[:],
                    lhsT=WkH[:, cc, :],
                    rhs=xT[:, cc, ts(tc, 512)],
                    start=(cc == 0), stop=(cc == 6),
                )
            nc.vector.tensor_copy(KT[:, ts(tc, 512)], kp[:])
            for tsub in range(4):
                kc = 4 * tc + tsub
                vp = mxp.tile([P, P], F32, tag="mx", name="vp")
                for cc in range(7):
                    nc.tensor.matmul(
                        vp[:],
                        lhsT=xT[:, cc, ts(kc, P)],
                        rhs=WvH[:, cc, :],
                        start=(cc == 0), stop=(cc == 6),
                    )
                nc.vector.tensor_copy(
                    V[:, kc, :, 0:D], vp.rearrange("p (g e) -> p g e", g=4)
                )

            # ---- attention for q-block tc over all local heads; AV chains
            # trail one head behind the score/exp stream to keep PE fed
            AO = aop.tile([P, 4, DH], F16, tag="AO")
            prev_pts = None
            for h in range(HL):
                pts = head_block(AO, h, tc)
                if prev_pts is not None:
                    emit_av(AO, h - 1, tc, prev_pts)
                prev_pts = pts
            emit_av(AO, HL - 1, tc, prev_pts)

            # ---- AO transpose + out-projection for this round
            for c in range(4):
                M = P if c < 3 else 64
                for j in range(4):
                    tr = mxp.tile([P, P], F16, tag="mx", name="tr")
                    nc.tensor.transpose(
                        tr[0:M, :], AO[:, j, c * P:c * P + M], identh[:]
                    )
                    nc.vector.tensor_copy(
                        AOT[0:M, c, tc * 512 + j * P:tc * 512 + (j + 1) * P],
                        tr[0:M, :],
                    )
            for tsub in range(4):
                tg = 4 * tc + tsub
                ob = obp.tile([P, C], F16, tag="ob")
                for ncol in range(2):
                    po = mxp.tile([P, DH], F32, tag="mx", name="po")
                    for c in range(4):
                        K = P if c < 3 else 64
                        nc.tensor.matmul(
                            po[:],
                            lhsT=AOT[0:K, c, ts(tg, P)],
                            rhs=WoH[0:K, c, ncol * DH:(ncol + 1) * DH],
                            start=(c == 0), stop=(c == 3),
                        )
                    nc.vector.tensor_copy(ob[:, ncol * DH:(ncol + 1) * DH], po[:])
                nc.sync.dma_start(ov[:, tg, :], ob[:])

        if "xt_d" in d:  # debug dumps (only present in debug builds)
            nc.sync.dma_start(d["xt_d"].rearrange("p (a b) -> p a b", a=7), xT[:])
            nc.sync.dma_start(d["qt_d"].rearrange("p (a b) -> p a b", a=4), QT[:])
            nc.sync.dma_start(d["kt_d"], KT[:])
            nc.sync.dma_start(
                d["v_d"].rearrange("p (a b c) -> p a b c", a=16, b=4), V[:]
            )
            nc.sync.dma_start(d["aot_d"].rearrange("p (a b) -> p a b", a=4), AOT[:])


_NC_CACHE = None


def _build():
    global _NC_CACHE
    if _NC_CACHE is not None:
        return _NC_CACHE
    nc = bacc.Bacc("TRN2", target_bir_lowering=False, debug=False, num_devices=8)
    d = {
        "x": nc.dram_tensor("x", (T, C), F16, kind="ExternalInput"),
        "wq": nc.dram_tensor("wq", (C, DH), F16, kind="ExternalInput"),
        "wk": nc.dram_tensor("wk", (C, P), F16, kind="ExternalInput"),
        "wv": nc.dram_tensor("wv", (C, P), F16, kind="ExternalInput"),
        "wo": nc.dram_tensor("wo", (DH, C), F16, kind="ExternalInput"),
        "identh": nc.dram_tensor("identh", (P, P), F16, kind="ExternalInput"),
        "maskc": nc.dram_tensor("maskc", (P, P), F16, kind="ExternalInput"),
        "out": nc.dram_tensor("out", (T, C), F16, kind="ExternalOutput"),
    }
    with tile.TileContext(nc) as tc_:
        _trace(tc_, {k: v[:] for k, v in d.items()})
    nc.compile()
    _NC_CACHE = nc
    return nc


def _in_maps(x, Wq, Wk, Wv, Wo):
    identh = np.eye(P, dtype=np.float16)
    # maskc[p, j] = MASKVAL where q-local j < kpos-local p (strict causal mask)
    maskc = np.where(
        np.arange(P)[None, :] < np.arange(P)[:, None], MASKVAL, 0.0
    ).astype(np.float16)
    maps = []
    for core in range(8):
        b, hf = core // 2, core % 2
        # Wq cols: slot-major [s, g, d] ordering
        qcols = []
        for s in range(4):
            for g in range(4 if s < 2 else 3):
                hloc = g * 4 + s if g < 3 else 12 + s
                H = HEADS_HALF[hf][hloc]
                qcols.extend(range(32 * H, 32 * H + 32))
        # Wk/Wv cols: group-major [g, d]
        kcols = np.concatenate(
            [np.arange(32 * kv, 32 * kv + 32) for kv in KV_HALF[hf]]
        )
        # Wo rows: local-head-major [h, d]
        orows = np.concatenate(
            [np.arange(32 * H, 32 * H + 32) for H in HEADS_HALF[hf]]
        )
        maps.append(
            {
                "x": np.ascontiguousarray(x[b]).astype(np.float16),
                "wq": np.ascontiguousarray(Wq[:, qcols]).astype(np.float16),
                "wk": np.ascontiguousarray(Wk[:, kcols]).astype(np.float16),
                "wv": np.ascontiguousarray(Wv[:, kcols]).astype(np.float16),
                "wo": np.ascontiguousarray(Wo[orows, :]).astype(np.float16),
                "identh": identh,
                "maskc": maskc,
            }
        )
    return maps


def run(x, Wq, Wk, Wv, Wo, trace=False):
    nc = _build()
    res = run_bass_kernel_spmd(
        nc, _in_maps(x, Wq, Wk, Wv, Wo), core_ids=list(range(8)), trace=trace
    )
    outs = [r["out"] for r in res.results]
    final = np.empty((4, T, C), np.float32)
    for b in range(4):
        final[b] = outs[2 * b].astype(np.float32) + outs[2 * b + 1].astype(
            np.float32
        )
    return final, res


def kernel(x, Wq, Wk, Wv, Wo):
    x = np.asarray(x, dtype=np.float32)
    out, _ = run(
        x,
        np.asarray(Wq, np.float32),
        np.asarray(Wk, np.float32),
        np.asarray(Wv, np.float32),
        np.asarray(Wo, np.float32),
    )
    return out


# revision 11
# speedup vs baseline: 1.7018x; 1.2044x over previous
"""Causal GQA self-attention on 8 Trainium2 NeuronCores (fp16 pipeline).

Sharding: data-parallel over batch (4) x tensor-parallel over heads (2 halves
of 14 heads each, KV heads replicated per GQA group). Each core computes its
heads' partial contribution through the row-parallel out-projection; the host
sums the two fp16 partials per batch element in fp32.

Per-core local structure: 4 local kv groups g (sizes 4,4,4,2 heads), local
head h -> (g = h//4, s = h%4 slot). All tensors fp16 except PSUM (f32).

Layouts (SBUF [128 partitions, free...]):
  X16 [128, 16, 896]   x rows (t%128 on partitions)
  xT  [128, 7, 2048]   x^T (C on partitions) via PE transpose
  QT  [128, 4, 2048]   Q^T: head (g,s) at partitions 32g..32g+32, slot s
  KT  [128, 2048]      K^T: group g at partitions 32g..32g+32
  V   [128, 16, 4, 33] V rows (kpos%128 on partitions), col 32 = ones (Z)
  AO  [128, 4, 448]    attn out rows (q%128 on partitions) per 512-q round
  AOT [128, 4, 2048]   attn out transposed (head dims on partitions)

Pipeline: 4 rounds of 512 tokens; per round: x-transpose + Q/K/V projection,
then per head: S^T = K^T.T @ Q^T per 128-kpos chunk (diagonal chunks get a
-180 mask preloaded into PSUM via an extra matmul, then accumulate), exp on
ScalarE (or Schraudolph fast-exp on DVE for a fraction of chunks: bit-trick
y = s*A+B -> int16 -> reinterpret as fp16), then AV flipped: out[q,d] with
P^T chunk as stationary operand and [V | 1] as 33-wide moving operand so the
softmax denominator Z rides along as column 32. Normalize on DVE, transpose
AO via PE, row-parallel out-projection, fp16 partial out.
"""

import sys

sys.path.insert(0, "/opt/trn_rl_repo")

import numpy as np

import concourse.bass as bass
import concourse.mybir as mybir
import concourse.tile as tile
from concourse import bacc
from concourse.bass import ts
from concourse.bass_utils import run_bass_kernel_spmd

F32 = mybir.dt.float32
F16 = mybir.dt.float16
I16 = mybir.dt.int16
EXP = mybir.ActivationFunctionType.Exp
MULT = mybir.AluOpType.mult
ADD = mybir.AluOpType.add
P = 128
T, C = 2048, 896
D = 32
HL = 14          # local heads per core
DH = HL * D      # 448
SCALE = 1.0 / float(np.sqrt(D))
MASKVAL = -180.0
# Schraudolph fast-exp consts (fp16 bit trick): y = s*A + B as int16
A_S = SCALE * 1024.0 / float(np.log(2.0))
B_S = 15.0 * 1024.0 - 0.043 * 1024.0
# every SCHR_MOD-th exp pair runs on DVE via Schraudolph (0 = all exact/ACT)
SCHR_MOD = 4

SOFF = [0, 128, 256, 352]   # Wq col offset per slot
SLOTW = [128, 128, 96, 96]  # slot widths (s>=2 lack group 3)

HEADS_HALF = [
    list(range(0, 12)) + [24, 25],
    list(range(12, 24)) + [26, 27],
]
KV_HALF = [[0, 1, 2, 6], [3, 4, 5, 6]]


def _head_gs(h):
    return (h // 4, h % 4) if h < 12 else (3, h - 12)


def _trace(tc_, d):
    nc = tc_.nc
    pair_ctr = [0]

    with tc_.tile_pool(name="const", bufs=1) as const, \
         tc_.tile_pool(name="persist", bufs=1) as persist, \
         tc_.tile_pool(name="aop", bufs=2) as aop, \
         tc_.tile_pool(name="ptp", bufs=18) as ptp, \
         tc_.tile_pool(name="rzp", bufs=2) as rzp, \
         tc_.tile_pool(name="obp", bufs=2) as obp, \
         tc_.tile_pool(name="spp", bufs=2, space="PSUM") as spp, \
         tc_.tile_pool(name="avp", bufs=2, space="PSUM") as avp, \
         tc_.tile_pool(name="mxp", bufs=2, space="PSUM") as mxp:

        identh = const.tile([P, P], F16)
        nc.sync.dma_start(identh[:], d["identh"][:])
        maskc = const.tile([P, P], F16)
        nc.sync.dma_start(maskc[:], d["maskc"][:])

        X16 = persist.tile([P, 16, C], F16, tag="X16")
        xT = persist.tile([P, 7, T], F16, tag="xT")
        QT = persist.tile([P, 4, T], F16, tag="QT")
        KT = persist.tile([P, T], F16, tag="KT")
        V = persist.tile([P, 16, 4, 33], F16, tag="V")
        AOT = persist.tile([P, 4, T], F16, tag="AOT")
        WqH = persist.tile([P, 7, DH], F16, tag="WqH")
        WkH = persist.tile([P, 7, P], F16, tag="WkH")
        WvH = persist.tile([P, 7, P], F16, tag="WvH")
        WoH = persist.tile([P, 4, C], F16, tag="WoH")

        xv = d["x"].rearrange("(to ti) c -> ti to c", ti=P)
        ov = d["out"].rearrange("(to ti) c -> ti to c", ti=P)

        # input DMAs: round-0 x first, then QKV weights, rest of x, Wo
        nc.sync.dma_start(X16[:, 0:4, :], xv[:, 0:4, :])
        nc.sync.dma_start(WqH[:], d["wq"].rearrange("(co ci) n -> ci co n", ci=P))
        nc.sync.dma_start(WkH[:], d["wk"].rearrange("(co ci) n -> ci co n", ci=P))
        nc.sync.dma_start(WvH[:], d["wv"].rearrange("(co ci) n -> ci co n", ci=P))
        for r in range(1, 4):
            nc.sync.dma_start(X16[:, 4 * r:4 * r + 4, :], xv[:, 4 * r:4 * r + 4, :])
        nc.sync.dma_start(
            WoH[:, :3, :], d["wo"][:3 * P, :].rearrange("(co ci) n -> ci co n", ci=P)
        )
        nc.sync.dma_start(WoH[:64, 3, :], d["wo"][3 * P:, :])
        nc.gpsimd.memset(V[:, :, :, 32:33], 1.0)

        def emit_av(AO, h, qc, pts):
            # one contiguous accumulation chain per q-subblock j (PSUM banks
            # support a single open matmul accumulation group at a time)
            g, _ = _head_gs(h)
            for j in range(4):
                av = avp.tile([P, 33], F32, tag="av", name="av")
                for ki in range(4 * qc + j + 1):
                    nc.tensor.matmul(
                        av[:, 0:33],
                        lhsT=pts[ki // 2][:, ki % 2, ts(j, P)],
                        rhs=V[:, ki, g, 0:33],
                        start=(ki == 0),
                        stop=(ki == 4 * qc + j),
                        skip_group_check=True,
                    )
                rz = rzp.tile([P, 1], F32, tag="rz", name="rz")
                nc.vector.reciprocal_approx_fast(rz[:], av[:, 32:33])
                nc.vector.tensor_tensor(
                    AO[:, j, h * D:(h + 1) * D],
                    av[:, 0:D],
                    rz.to_broadcast((P, D)),
                    MULT,
                )

        def head_block(AO, h, qc):
            g, s = _head_gs(h)
            qs = qc * 512
            npair = 2 * qc + 2
            pts = []
            for p in range(npair):
                sp = spp.tile([P, 2, 512], F32, tag="sp")
                for sl in range(2):
                    ki = 2 * p + sl
                    ks = ki * P
                    dg = ki - 4 * qc
                    if dg < 0:
                        nc.tensor.matmul(
                            sp[:, sl, :],
                            lhsT=KT[ts(g, D), ks:ks + P],
                            rhs=QT[ts(g, D), s, qs:qs + 512],
                            start=True, stop=True, skip_group_check=True,
                            tile_position=(g * D, 0),
                        )
                    else:
                        qoff = P * dg
                        nc.tensor.matmul(
                            sp[:, sl, qoff:qoff + P],
                            lhsT=identh[:],
                            rhs=maskc[:],
                            start=True, stop=False, skip_group_check=True,
                        )
                        nc.tensor.matmul(
                            sp[:, sl, qoff:qoff + P],
                            lhsT=KT[ts(g, D), ks:ks + P],
                            rhs=QT[ts(g, D), s, qs + qoff:qs + qoff + P],
                            start=False, stop=True, skip_group_check=True,
                            tile_position=(g * D, 0),
                        )
                        if qoff + P < 512:
                            nc.tensor.matmul(
                                sp[:, sl, qoff + P:512],
                                lhsT=KT[ts(g, D), ks:ks + P],
                                rhs=QT[ts(g, D), s, qs + qoff + P:qs + 512],
                                start=True, stop=True, skip_group_check=True,
                                tile_position=(g * D, 0),
                            )
                qoffE = P * max(0, 2 * p - 4 * qc)
                pt = ptp.tile([P, 2, 512], F16, tag="pt")
                pair_ctr[0] += 1
                if SCHR_MOD and pair_ctr[0] % SCHR_MOD == 0:
                    nc.vector.tensor_scalar(
                        pt[:, :, qoffE:512].bitcast(I16),
                        sp[:, :, qoffE:512],
                        A_S, B_S, MULT, ADD,
                    )
                else:
                    nc.scalar.activation(
                        pt[:, :, qoffE:512], sp[:, :, qoffE:512], EXP, scale=SCALE
                    )
                pts.append(pt)
            return pts

        def prep_pieces(tc):
            # x transpose + Q/K/V projections for round tc, as closures so
            # they can be interleaved into the previous round's head stream
            def xtr(cc):
                def f():
                    pst = mxp.tile([P, 512], F16, tag="mx", name="pst")
                    for k2 in range(4):
                        nc.tensor.transpose(
                            pst[:, ts(k2, P)],
                            X16[:, 4 * tc + k2, ts(cc, P)],
                            identh[:],
                        )
                    nc.vector.tensor_copy(xT[:, cc, ts(tc, 512)], pst[:])
                return f

            def qproj(s):
                def f():
                    Ms = SLOTW[s]
                    qp = mxp.tile([P, 512], F32, tag="mx", name="qp")
                    for cc in range(7):
                        nc.tensor.matmul(
                            qp[0:Ms, :],
                            lhsT=WqH[:, cc, SOFF[s]:SOFF[s] + Ms],
                            rhs=xT[:, cc, ts(tc, 512)],
                            start=(cc == 0), stop=(cc == 6),
                        )
                    nc.vector.tensor_copy(QT[0:Ms, s, ts(tc, 512)], qp[0:Ms, :])
                return f

            def kproj():
                kp = mxp.tile([P, 512], F32, tag="mx", name="kp")
                for cc in range(7):
                    nc.tensor.matmul(
                        kp[:],
                        lhsT=WkH[:, cc, :],
                        rhs=xT[:, cc, ts(tc, 512)],
                        start=(cc == 0), stop=(cc == 6),
                    )
                nc.vector.tensor_copy(KT[:, ts(tc, 512)], kp[:])

            def vproj(tsub):
                def f():
                    kc = 4 * tc + tsub
                    vp = mxp.tile([P, P], F32, tag="mx", name="vp")
                    for cc in range(7):
                        nc.tensor.matmul(
                            vp[:],
                            lhsT=xT[:, cc, ts(kc, P)],
                            rhs=WvH[:, cc, :],
                            start=(cc == 0), stop=(cc == 6),
                        )
                    nc.vector.tensor_copy(
                        V[:, kc, :, 0:D], vp.rearrange("p (g e) -> p g e", g=4)
                    )
                return f

            return (
                [xtr(cc) for cc in range(7)]
                + [qproj(s) for s in range(4)]
                + [kproj]
                + [vproj(t) for t in range(4)]
            )

        def posts_pieces(tc, AO):
            # AO transpose + out-projection for round tc
            def aotr(c):
                def f():
                    M = P if c < 3 else 64
                    for j in range(4):
                        tr = mxp.tile([P, P], F16, tag="mx", name="tr")
                        nc.tensor.transpose(
                            tr[0:M, :], AO[:, j, c * P:c * P + M], identh[:]
                        )
                        nc.vector.tensor_copy(
                            AOT[0:M, c, tc * 512 + j * P:tc * 512 + (j + 1) * P],
                            tr[0:M, :],
                        )
                return f

            def oproj(tsub):
                def f():
                    tg = 4 * tc + tsub
                    ob = obp.tile([P, C], F16, tag="ob")
                    for ncol in range(2):
                        po = mxp.tile([P, DH], F32, tag="mx", name="po")
                        for c in range(4):
                            K = P if c < 3 else 64
                            nc.tensor.matmul(
                                po[:],
                                lhsT=AOT[0:K, c, ts(tg, P)],
                                rhs=WoH[0:K, c, ncol * DH:(ncol + 1) * DH],
                                start=(c == 0), stop=(c == 3),
                            )
                        nc.vector.tensor_copy(
                            ob[:, ncol * DH:(ncol + 1) * DH], po[:]
                        )
                    nc.sync.dma_start(ov[:, tg, :], ob[:])
                return f

            return [aotr(c) for c in range(4)] + [oproj(t) for t in range(4)]

        for piece in prep_pieces(0):
            piece()
        prevAO = None
        for tc in range(4):
            # pending work interleaved into this round's head stream: next
            # round's projections + previous round's output projection
            pend = []
            if tc < 3:
                pend += prep_pieces(tc + 1)
            if prevAO is not None:
                pend += posts_pieces(tc - 1, prevAO)
            AO = aop.tile([P, 4, DH], F16, tag="AO")
            prev_pts = None
            for h in range(HL):
                pts = head_block(AO, h, tc)
                if prev_pts is not None:
                    emit_av(AO, h - 1, tc, prev_pts)
                prev_pts = pts
                for _ in range(2):
                    if pend:
                        pend.pop(0)()
            emit_av(AO, HL - 1, tc, prev_pts)
            for piece in pend:
                piece()
            prevAO = AO
        for piece in posts_pieces(3, prevAO):
            piece()

        if "xt_d" in d:  # debug dumps (only present in debug builds)
            nc.sync.dma_start(d["xt_d"].rearrange("p (a b) -> p a b", a=7), xT[:])
            nc.sync.dma_start(d["qt_d"].rearrange("p (a b) -> p a b", a=4), QT[:])
            nc.sync.dma_start(d["kt_d"], KT[:])
            nc.sync.dma_start(
                d["v_d"].rearrange("p (a b c) -> p a b c", a=16, b=4), V[:]
            )
            nc.sync.dma_start(d["aot_d"].rearrange("p (a b) -> p a b", a=4), AOT[:])


_NC_CACHE = None


def _build():
    global _NC_CACHE
    if _NC_CACHE is not None:
        return _NC_CACHE
    nc = bacc.Bacc("TRN2", target_bir_lowering=False, debug=False, num_devices=8)
    d = {
        "x": nc.dram_tensor("x", (T, C), F16, kind="ExternalInput"),
        "wq": nc.dram_tensor("wq", (C, DH), F16, kind="ExternalInput"),
        "wk": nc.dram_tensor("wk", (C, P), F16, kind="ExternalInput"),
        "wv": nc.dram_tensor("wv", (C, P), F16, kind="ExternalInput"),
        "wo": nc.dram_tensor("wo", (DH, C), F16, kind="ExternalInput"),
        "identh": nc.dram_tensor("identh", (P, P), F16, kind="ExternalInput"),
        "maskc": nc.dram_tensor("maskc", (P, P), F16, kind="ExternalInput"),
        "out": nc.dram_tensor("out", (T, C), F16, kind="ExternalOutput"),
    }
    with tile.TileContext(nc) as tc_:
        _trace(tc_, {k: v[:] for k, v in d.items()})
    nc.compile()
    _NC_CACHE = nc
    return nc


def _in_maps(x, Wq, Wk, Wv, Wo):
    identh = np.eye(P, dtype=np.float16)
    # maskc[p, j] = MASKVAL where q-local j < kpos-local p (strict causal mask)
    maskc = np.where(
        np.arange(P)[None, :] < np.arange(P)[:, None], MASKVAL, 0.0
    ).astype(np.float16)
    maps = []
    for core in range(8):
        b, hf = core // 2, core % 2
        # Wq cols: slot-major [s, g, d] ordering
        qcols = []
        for s in range(4):
            for g in range(4 if s < 2 else 3):
                hloc = g * 4 + s if g < 3 else 12 + s
                H = HEADS_HALF[hf][hloc]
                qcols.extend(range(32 * H, 32 * H + 32))
        # Wk/Wv cols: group-major [g, d]
        kcols = np.concatenate(
            [np.arange(32 * kv, 32 * kv + 32) for kv in KV_HALF[hf]]
        )
        # Wo rows: local-head-major [h, d]
        orows = np.concatenate(
            [np.arange(32 * H, 32 * H + 32) for H in HEADS_HALF[hf]]
        )
        maps.append(
            {
                "x": np.ascontiguousarray(x[b]).astype(np.float16),
                "wq": np.ascontiguousarray(Wq[:, qcols]).astype(np.float16),
                "wk": np.ascontiguousarray(Wk[:, kcols]).astype(np.float16),
                "wv": np.ascontiguousarray(Wv[:, kcols]).astype(np.float16),
                "wo": np.ascontiguousarray(Wo[orows, :]).astype(np.float16),
                "identh": identh,
                "maskc": maskc,
            }
        )
    return maps


def run(x, Wq, Wk, Wv, Wo, trace=False):
    nc = _build()
    res = run_bass_kernel_spmd(
        nc, _in_maps(x, Wq, Wk, Wv, Wo), core_ids=list(range(8)), trace=trace
    )
    outs = [r["out"] for r in res.results]
    final = np.empty((4, T, C), np.float32)
    for b in range(4):
        final[b] = outs[2 * b].astype(np.float32) + outs[2 * b + 1].astype(
            np.float32
        )
    return final, res


def kernel(x, Wq, Wk, Wv, Wo):
    x = np.asarray(x, dtype=np.float32)
    out, _ = run(
        x,
        np.asarray(Wq, np.float32),
        np.asarray(Wk, np.float32),
        np.asarray(Wv, np.float32),
        np.asarray(Wo, np.float32),
    )
    return out


# revision 13
# speedup vs baseline: 1.8395x; 1.0809x over previous
"""Causal GQA self-attention on 8 Trainium2 NeuronCores (fp16 pipeline).

Sharding: data-parallel over batch (4) x tensor-parallel over heads (2 halves
of 14 heads each, KV heads replicated per GQA group). Each core computes its
heads' partial contribution through the row-parallel out-projection; the host
sums the two fp16 partials per batch element in fp32.

Per-core local structure: 4 local kv groups g (sizes 4,4,4,2 heads), local
head h -> (g = h//4, s = h%4 slot). All tensors fp16 except PSUM (f32).

Layouts (SBUF [128 partitions, free...]):
  X16 [128, 16, 896]   x rows (t%128 on partitions)
  xT  [128, 7, 2048]   x^T (C on partitions) via PE transpose
  QT  [128, 4, 2048]   Q^T: head (g,s) at partitions 32g..32g+32, slot s
  KT  [128, 2048]      K^T: group g at partitions 32g..32g+32
  V   [128, 16, 4, 33] V rows (kpos%128 on partitions), col 32 = ones (Z)
  AO  [128, 4, 448]    attn out rows (q%128 on partitions) per 512-q round
  AOT [128, 4, 2048]   attn out transposed (head dims on partitions)

Pipeline: 4 rounds of 512 tokens; per round: x-transpose + Q/K/V projection,
then per head: S^T = K^T.T @ Q^T per 128-kpos chunk (diagonal chunks get a
-180 mask preloaded into PSUM via an extra matmul, then accumulate), exp on
ScalarE (or Schraudolph fast-exp on DVE for a fraction of chunks: bit-trick
y = s*A+B -> int16 -> reinterpret as fp16), then AV flipped: out[q,d] with
P^T chunk as stationary operand and [V | 1] as 33-wide moving operand so the
softmax denominator Z rides along as column 32. Normalize on DVE, transpose
AO via PE, row-parallel out-projection, fp16 partial out.
"""

import sys

sys.path.insert(0, "/opt/trn_rl_repo")

import numpy as np

import concourse.bass as bass
import concourse.mybir as mybir
import concourse.tile as tile
from concourse import bacc
from concourse.bass import ts
from concourse.bass_utils import run_bass_kernel_spmd

F32 = mybir.dt.float32
F16 = mybir.dt.float16
I16 = mybir.dt.int16
EXP = mybir.ActivationFunctionType.Exp
MULT = mybir.AluOpType.mult
ADD = mybir.AluOpType.add
P = 128
T, C = 2048, 896
D = 32
HL = 14          # local heads per core
DH = HL * D      # 448
SCALE = 1.0 / float(np.sqrt(D))
MASKVAL = -180.0
# Schraudolph fast-exp consts (fp16 bit trick): y = s*A + B as int16
A_S = SCALE * 1024.0 / float(np.log(2.0))
B_S = 15.0 * 1024.0 - 0.043 * 1024.0
# every SCHR_MOD-th exp pair runs on DVE via Schraudolph (0 = all exact/ACT)
SCHR_MOD = 3

SOFF = [0, 128, 256, 352]   # Wq col offset per slot
SLOTW = [128, 128, 96, 96]  # slot widths (s>=2 lack group 3)

HEADS_HALF = [
    list(range(0, 12)) + [24, 25],
    list(range(12, 24)) + [26, 27],
]
KV_HALF = [[0, 1, 2, 6], [3, 4, 5, 6]]


def _head_gs(h):
    return (h // 4, h % 4) if h < 12 else (3, h - 12)


def _trace(tc_, d):
    nc = tc_.nc
    pair_ctr = [0]

    with tc_.tile_pool(name="const", bufs=1) as const, \
         tc_.tile_pool(name="persist", bufs=1) as persist, \
         tc_.tile_pool(name="aop", bufs=2) as aop, \
         tc_.tile_pool(name="ptp", bufs=18) as ptp, \
         tc_.tile_pool(name="rzp", bufs=2) as rzp, \
         tc_.tile_pool(name="obp", bufs=2) as obp, \
         tc_.tile_pool(name="spp", bufs=3, space="PSUM") as spp, \
         tc_.tile_pool(name="avp", bufs=2, space="PSUM") as avp:

        identh = const.tile([P, P], F16)
        nc.sync.dma_start(identh[:], d["identh"][:])
        maskc = const.tile([P, P], F16)
        nc.sync.dma_start(maskc[:], d["maskc"][:])

        X16 = persist.tile([P, 16, C], F16, tag="X16")
        xT = persist.tile([P, 7, T], F16, tag="xT")
        QT = persist.tile([P, 4, T], F16, tag="QT")
        KT = persist.tile([P, T], F16, tag="KT")
        V = persist.tile([P, 16, 4, 33], F16, tag="V")
        AOT = persist.tile([P, 4, T], F16, tag="AOT")
        WqH = persist.tile([P, 7, DH], F16, tag="WqH")
        WkH = persist.tile([P, 7, P], F16, tag="WkH")
        WvH = persist.tile([P, 7, P], F16, tag="WvH")
        WoH = persist.tile([P, 4, C], F16, tag="WoH")

        xv = d["x"].rearrange("(to ti) c -> ti to c", ti=P)
        ov = d["out"].rearrange("(to ti) c -> ti to c", ti=P)

        # input DMAs: round-0 x first, then QKV weights, rest of x, Wo
        nc.sync.dma_start(X16[:, 0:4, :], xv[:, 0:4, :])
        nc.sync.dma_start(WqH[:], d["wq"].rearrange("(co ci) n -> ci co n", ci=P))
        nc.sync.dma_start(WkH[:], d["wk"].rearrange("(co ci) n -> ci co n", ci=P))
        nc.sync.dma_start(WvH[:], d["wv"].rearrange("(co ci) n -> ci co n", ci=P))
        for r in range(1, 4):
            nc.sync.dma_start(X16[:, 4 * r:4 * r + 4, :], xv[:, 4 * r:4 * r + 4, :])
        nc.sync.dma_start(
            WoH[:, :3, :], d["wo"][:3 * P, :].rearrange("(co ci) n -> ci co n", ci=P)
        )
        nc.sync.dma_start(WoH[:64, 3, :], d["wo"][3 * P:, :])
        nc.gpsimd.memset(V[:, :, :, 32:33], 1.0)

        def emit_av(AO, h, qc, pts):
            # one contiguous accumulation chain per q-subblock j (PSUM banks
            # support a single open matmul accumulation group at a time)
            g, _ = _head_gs(h)
            for j in range(4):
                av = avp.tile([P, 33], F32, tag="av", name="av")
                for ki in range(4 * qc + j + 1):
                    nc.tensor.matmul(
                        av[:, 0:33],
                        lhsT=pts[ki // 2][:, ki % 2, ts(j, P)],
                        rhs=V[:, ki, g, 0:33],
                        start=(ki == 0),
                        stop=(ki == 4 * qc + j),
                        skip_group_check=True,
                    )
                rz = rzp.tile([P, 1], F32, tag="rz", name="rz")
                nc.vector.reciprocal_approx_fast(rz[:], av[:, 32:33])
                nc.vector.tensor_tensor(
                    AO[:, j, h * D:(h + 1) * D],
                    av[:, 0:D],
                    rz.to_broadcast((P, D)),
                    MULT,
                )

        def head_block(AO, h, qc):
            g, s = _head_gs(h)
            qs = qc * 512
            npair = 2 * qc + 2
            pts = []
            for p in range(npair):
                sp = spp.tile([P, 2, 512], F32, tag="sp")
                for sl in range(2):
                    ki = 2 * p + sl
                    ks = ki * P
                    dg = ki - 4 * qc
                    if dg < 0:
                        nc.tensor.matmul(
                            sp[:, sl, :],
                            lhsT=KT[ts(g, D), ks:ks + P],
                            rhs=QT[ts(g, D), s, qs:qs + 512],
                            start=True, stop=True, skip_group_check=True,
                            tile_position=(g * D, 0),
                        )
                    else:
                        qoff = P * dg
                        nc.tensor.matmul(
                            sp[:, sl, qoff:qoff + P],
                            lhsT=identh[:],
                            rhs=maskc[:],
                            start=True, stop=False, skip_group_check=True,
                        )
                        nc.tensor.matmul(
                            sp[:, sl, qoff:qoff + P],
                            lhsT=KT[ts(g, D), ks:ks + P],
                            rhs=QT[ts(g, D), s, qs + qoff:qs + qoff + P],
                            start=False, stop=True, skip_group_check=True,
                            tile_position=(g * D, 0),
                        )
                        if qoff + P < 512:
                            nc.tensor.matmul(
                                sp[:, sl, qoff + P:512],
                                lhsT=KT[ts(g, D), ks:ks + P],
                                rhs=QT[ts(g, D), s, qs + qoff + P:qs + 512],
                                start=True, stop=True, skip_group_check=True,
                                tile_position=(g * D, 0),
                            )
                qoffE = P * max(0, 2 * p - 4 * qc)
                pt = ptp.tile([P, 2, 512], F16, tag="pt")
                pair_ctr[0] += 1
                if SCHR_MOD and pair_ctr[0] % SCHR_MOD == 0:
                    nc.vector.tensor_scalar(
                        pt[:, :, qoffE:512].bitcast(I16),
                        sp[:, :, qoffE:512],
                        A_S, B_S, MULT, ADD,
                    )
                else:
                    nc.scalar.activation(
                        pt[:, :, qoffE:512], sp[:, :, qoffE:512], EXP, scale=SCALE
                    )
                pts.append(pt)
            return pts

        def prep_pieces(tc):
            # x transpose + Q/K/V projections for round tc, as closures so
            # they can be interleaved into the previous round's head stream
            def xtr(cc):
                def f():
                    pst = spp.tile([P, 512], F16, tag="sp", name="pst")
                    for k2 in range(4):
                        nc.tensor.transpose(
                            pst[:, ts(k2, P)],
                            X16[:, 4 * tc + k2, ts(cc, P)],
                            identh[:],
                        )
                    nc.vector.tensor_copy(xT[:, cc, ts(tc, 512)], pst[:])
                return f

            def qproj(s):
                def f():
                    Ms = SLOTW[s]
                    qp = spp.tile([P, 512], F32, tag="sp", name="qp")
                    for cc in range(7):
                        nc.tensor.matmul(
                            qp[0:Ms, :],
                            lhsT=WqH[:, cc, SOFF[s]:SOFF[s] + Ms],
                            rhs=xT[:, cc, ts(tc, 512)],
                            start=(cc == 0), stop=(cc == 6),
                        )
                    nc.vector.tensor_copy(QT[0:Ms, s, ts(tc, 512)], qp[0:Ms, :])
                return f

            def kproj():
                kp = spp.tile([P, 512], F32, tag="sp", name="kp")
                for cc in range(7):
                    nc.tensor.matmul(
                        kp[:],
                        lhsT=WkH[:, cc, :],
                        rhs=xT[:, cc, ts(tc, 512)],
                        start=(cc == 0), stop=(cc == 6),
                    )
                nc.vector.tensor_copy(KT[:, ts(tc, 512)], kp[:])

            def vproj(tsub):
                def f():
                    kc = 4 * tc + tsub
                    vp = spp.tile([P, P], F32, tag="sp", name="vp")
                    for cc in range(7):
                        nc.tensor.matmul(
                            vp[:],
                            lhsT=xT[:, cc, ts(kc, P)],
                            rhs=WvH[:, cc, :],
                            start=(cc == 0), stop=(cc == 6),
                        )
                    nc.vector.tensor_copy(
                        V[:, kc, :, 0:D], vp.rearrange("p (g e) -> p g e", g=4)
                    )
                return f

            return (
                [xtr(cc) for cc in range(7)]
                + [qproj(s) for s in range(4)]
                + [kproj]
                + [vproj(t) for t in range(4)]
            )

        def posts_pieces(tc, AO):
            # AO transpose + out-projection for round tc
            def aotr(c):
                def f():
                    M = P if c < 3 else 64
                    for j in range(4):
                        tr = spp.tile([P, P], F16, tag="sp", name="tr")
                        nc.tensor.transpose(
                            tr[0:M, :], AO[:, j, c * P:c * P + M], identh[:]
                        )
                        nc.vector.tensor_copy(
                            AOT[0:M, c, tc * 512 + j * P:tc * 512 + (j + 1) * P],
                            tr[0:M, :],
                        )
                return f

            def oproj(tsub):
                def f():
                    tg = 4 * tc + tsub
                    ob = obp.tile([P, C], F16, tag="ob")
                    for ncol in range(2):
                        po = spp.tile([P, DH], F32, tag="sp", name="po")
                        for c in range(4):
                            K = P if c < 3 else 64
                            nc.tensor.matmul(
                                po[:],
                                lhsT=AOT[0:K, c, ts(tg, P)],
                                rhs=WoH[0:K, c, ncol * DH:(ncol + 1) * DH],
                                start=(c == 0), stop=(c == 3),
                            )
                        nc.vector.tensor_copy(
                            ob[:, ncol * DH:(ncol + 1) * DH], po[:]
                        )
                    nc.sync.dma_start(ov[:, tg, :], ob[:])
                return f

            return [aotr(c) for c in range(4)] + [oproj(t) for t in range(4)]

        for piece in prep_pieces(0):
            piece()
        prevAO = None
        for tc in range(4):
            # pending work interleaved into this round's head stream: next
            # round's projections + previous round's output projection
            pend = []
            if tc < 3:
                pend += prep_pieces(tc + 1)
            if prevAO is not None:
                pend += posts_pieces(tc - 1, prevAO)
            AO = aop.tile([P, 4, DH], F16, tag="AO")
            prev_pts = None
            for h in range(HL):
                pts = head_block(AO, h, tc)
                if prev_pts is not None:
                    emit_av(AO, h - 1, tc, prev_pts)
                prev_pts = pts
                for _ in range(2):
                    if pend:
                        pend.pop(0)()
            emit_av(AO, HL - 1, tc, prev_pts)
            for piece in pend:
                piece()
            prevAO = AO
        for piece in posts_pieces(3, prevAO):
            piece()

        if "xt_d" in d:  # debug dumps (only present in debug builds)
            nc.sync.dma_start(d["xt_d"].rearrange("p (a b) -> p a b", a=7), xT[:])
            nc.sync.dma_start(d["qt_d"].rearrange("p (a b) -> p a b", a=4), QT[:])
            nc.sync.dma_start(d["kt_d"], KT[:])
            nc.sync.dma_start(
                d["v_d"].rearrange("p (a b c) -> p a b c", a=16, b=4), V[:]
            )
            nc.sync.dma_start(d["aot_d"].rearrange("p (a b) -> p a b", a=4), AOT[:])


_NC_CACHE = None


def _build():
    global _NC_CACHE
    if _NC_CACHE is not None:
        return _NC_CACHE
    nc = bacc.Bacc("TRN2", target_bir_lowering=False, debug=False, num_devices=8)
    d = {
        "x": nc.dram_tensor("x", (T, C), F16, kind="ExternalInput"),
        "wq": nc.dram_tensor("wq", (C, DH), F16, kind="ExternalInput"),
        "wk": nc.dram_tensor("wk", (C, P), F16, kind="ExternalInput"),
        "wv": nc.dram_tensor("wv", (C, P), F16, kind="ExternalInput"),
        "wo": nc.dram_tensor("wo", (DH, C), F16, kind="ExternalInput"),
        "identh": nc.dram_tensor("identh", (P, P), F16, kind="ExternalInput"),
        "maskc": nc.dram_tensor("maskc", (P, P), F16, kind="ExternalInput"),
        "out": nc.dram_tensor("out", (T, C), F16, kind="ExternalOutput"),
    }
    with tile.TileContext(nc) as tc_:
        _trace(tc_, {k: v[:] for k, v in d.items()})
    nc.compile()
    _NC_CACHE = nc
    return nc


def _in_maps(x, Wq, Wk, Wv, Wo):
    identh = np.eye(P, dtype=np.float16)
    # maskc[p, j] = MASKVAL where q-local j < kpos-local p (strict causal mask)
    maskc = np.where(
        np.arange(P)[None, :] < np.arange(P)[:, None], MASKVAL, 0.0
    ).astype(np.float16)
    maps = []
    for core in range(8):
        b, hf = core // 2, core % 2
        # Wq cols: slot-major [s, g, d] ordering
        qcols = []
        for s in range(4):
            for g in range(4 if s < 2 else 3):
                hloc = g * 4 + s if g < 3 else 12 + s
                H = HEADS_HALF[hf][hloc]
                qcols.extend(range(32 * H, 32 * H + 32))
        # Wk/Wv cols: group-major [g, d]
        kcols = np.concatenate(
            [np.arange(32 * kv, 32 * kv + 32) for kv in KV_HALF[hf]]
        )
        # Wo rows: local-head-major [h, d]
        orows = np.concatenate(
            [np.arange(32 * H, 32 * H + 32) for H in HEADS_HALF[hf]]
        )
        maps.append(
            {
                "x": np.ascontiguousarray(x[b]).astype(np.float16),
                "wq": np.ascontiguousarray(Wq[:, qcols]).astype(np.float16),
                "wk": np.ascontiguousarray(Wk[:, kcols]).astype(np.float16),
                "wv": np.ascontiguousarray(Wv[:, kcols]).astype(np.float16),
                "wo": np.ascontiguousarray(Wo[orows, :]).astype(np.float16),
                "identh": identh,
                "maskc": maskc,
            }
        )
    return maps


def run(x, Wq, Wk, Wv, Wo, trace=False):
    nc = _build()
    res = run_bass_kernel_spmd(
        nc, _in_maps(x, Wq, Wk, Wv, Wo), core_ids=list(range(8)), trace=trace
    )
    outs = [r["out"] for r in res.results]
    final = np.empty((4, T, C), np.float32)
    for b in range(4):
        final[b] = outs[2 * b].astype(np.float32) + outs[2 * b + 1].astype(
            np.float32
        )
    return final, res


def kernel(x, Wq, Wk, Wv, Wo):
    x = np.asarray(x, dtype=np.float32)
    out, _ = run(
        x,
        np.asarray(Wq, np.float32),
        np.asarray(Wk, np.float32),
        np.asarray(Wv, np.float32),
        np.asarray(Wo, np.float32),
    )
    return out
